# revision 7
# baseline (speedup 1.0000x reference)
"""DotLoss kernel for Trainium2, data-parallel over 8 NeuronCores.

loss = mean_i[ relu(1 + dot(img[I[i]], aud[i]) - dot(img[i], aud[i]))
             + relu(1 + dot(img[i], aud[A[i]]) - dot(img[i], aud[i])) ]

Each core handles N/8 = 4096 rows: local rows stream in via contiguous
HWDGE DMAs, impostor rows via SWDGE dma_gather from the full (replicated)
embedding tables in device DRAM. Row dots are fused multiply+reduce ops on
the vector engine. Each core emits a [128,1] partial hinge-sum; the host
sums partials and divides by N.

Row->-(partition,slot) mapping: local chunk k row = k*CHUNK + p*SLOTS + c
lands at partition p, slot c (keeps local DMA fully contiguous per
partition). dma_gather position i lands at partition i%128, slot i//128, so
the host permutes each chunk's impostor indices with i = c*128 + p. The
summed loss is permutation-invariant, so only the triple alignment matters.
"""

import numpy as np

N, D = 32768, 512
NCORES = 8
SHARD = N // NCORES          # 4096 rows per core
P = 128
CHUNK = 512                  # rows per pipelined chunk
NCHUNK = SHARD // CHUNK
SLOTS = CHUNK // P           # [128, SLOTS, D] tiles per chunk
TSLOTS = SHARD // P          # accumulator columns per core
IC = CHUNK // 16             # idx columns per chunk in the wrapped layout

_CACHE = {}


def _build_nc():
    import concourse.bacc as bacc
    import concourse.mybir as mybir
    import concourse.tile as tile
    from contextlib import ExitStack

    fp32 = mybir.dt.float32
    i16 = mybir.dt.int16

    nc = bacc.Bacc("TRN2")
    img_full = nc.dram_tensor("img_full", [N, D], fp32, kind="ExternalInput")
    aud_full = nc.dram_tensor("aud_full", [N, D], fp32, kind="ExternalInput")
    img_loc = nc.dram_tensor("img_loc", [SHARD, D], fp32, kind="ExternalInput")
    aud_loc = nc.dram_tensor("aud_loc", [SHARD, D], fp32, kind="ExternalInput")
    iidx = nc.dram_tensor("iidx", [P, SHARD // 16], i16, kind="ExternalInput")
    aidx = nc.dram_tensor("aidx", [P, SHARD // 16], i16, kind="ExternalInput")
    partial = nc.dram_tensor("partial", [P, 1], fp32, kind="ExternalOutput")

    img_loc_r = img_loc.rearrange("(k p c) d -> k p c d", p=P, c=SLOTS)
    aud_loc_r = aud_loc.rearrange("(k p c) d -> k p c d", p=P, c=SLOTS)

    mult = mybir.AluOpType.mult
    add = mybir.AluOpType.add
    amax = mybir.AluOpType.max

    with ExitStack() as ctx:
        tc = ctx.enter_context(tile.TileContext(nc))
        io = ctx.enter_context(tc.tile_pool(name="io", bufs=3))
        idxp = ctx.enter_context(tc.tile_pool(name="idxp", bufs=1))
        acc = ctx.enter_context(tc.tile_pool(name="acc", bufs=1))
        scr = ctx.enter_context(tc.tile_pool(name="scr", bufs=6))

        iidx_sb = idxp.tile([P, SHARD // 16], i16, tag="iidx")
        nc.sync.dma_start(out=iidx_sb[:], in_=iidx[:])
        aidx_sb = idxp.tile([P, SHARD // 16], i16, tag="aidx")
        nc.sync.dma_start(out=aidx_sb[:], in_=aidx[:])

        anchor = acc.tile([P, TSLOTS], fp32, tag="anchor")
        iimp = acc.tile([P, TSLOTS], fp32, tag="iimp")
        aimp = acc.tile([P, TSLOTS], fp32, tag="aimp")

        for k in range(NCHUNK):
            li = io.tile([P, SLOTS, D], fp32, tag="li")
            nc.sync.dma_start(out=li[:], in_=img_loc_r[k])
            la = io.tile([P, SLOTS, D], fp32, tag="la")
            nc.sync.dma_start(out=la[:], in_=aud_loc_r[k])
            gi = io.tile([P, SLOTS, D], fp32, tag="gi")
            nc.gpsimd.dma_gather(
                out_ap=gi[:],
                in_ap=img_full[:],
                idxs_ap=iidx_sb[:, k * IC:(k + 1) * IC],
                num_idxs=CHUNK,
                num_idxs_reg=CHUNK,
                elem_size=D,
            )
            ga = io.tile([P, SLOTS, D], fp32, tag="ga")
            nc.gpsimd.dma_gather(
                out_ap=ga[:],
                in_ap=aud_full[:],
                idxs_ap=aidx_sb[:, k * IC:(k + 1) * IC],
                num_idxs=CHUNK,
                num_idxs_reg=CHUNK,
                elem_size=D,
            )
            for c in range(SLOTS):
                g = k * SLOTS + c
                for dst, a, b in ((anchor, li, la), (iimp, gi, la), (aimp, li, ga)):
                    pr = scr.tile([P, D], fp32, tag="pr")
                    nc.vector.scalar_tensor_tensor(
                        out=pr[:], in0=a[:, c], scalar=1.0, in1=b[:, c],
                        op0=mult, op1=mult,
                        accum_out=dst[:, g:g + 1],
                    )

        diff = acc.tile([P, 2 * TSLOTS], fp32, tag="diff")
        nc.vector.tensor_sub(diff[:, 0:TSLOTS], iimp[:], anchor[:])
        nc.vector.tensor_sub(diff[:, TSLOTS:], aimp[:], anchor[:])
        hout = acc.tile([P, 2 * TSLOTS], fp32, tag="hout")
        nc.vector.tensor_scalar(
            out=hout[:], in0=diff[:], scalar1=1.0, scalar2=0.0,
            op0=add, op1=amax,
        )
        psum_t = acc.tile([P, 1], fp32, tag="psum")
        nc.vector.tensor_reduce(
            out=psum_t[:], in_=hout[:], axis=mybir.AxisListType.X, op=add,
        )
        nc.sync.dma_start(out=partial[:], in_=psum_t[:])

    nc.compile()
    return nc


def _get_nc():
    if "nc" not in _CACHE:
        _CACHE["nc"] = _build_nc()
    return _CACHE["nc"]


def _prep_idx(imp_core):
    """Wrap one core's impostor indices into the dma_gather SBUF layout.

    Per chunk, gather position i = c*128 + p must fetch the impostor of
    local row p*SLOTS + c. The wrapped tile stores gather position j at
    [j % 16, j // 16], replicated across the 8 GPSIMD partition groups.
    """
    g = imp_core.reshape(NCHUNK, P, SLOTS)
    gi = np.transpose(g, (0, 2, 1)).reshape(NCHUNK, CHUNK)   # [k, c*P + p]
    w = gi.reshape(NCHUNK, IC, 16)
    w = np.transpose(w, (2, 0, 1)).reshape(16, SHARD // 16)  # [q, (k s)]
    return np.ascontiguousarray(np.tile(w, (8, 1)).astype(np.int16))


def make_in_maps(image_outputs, audio_outputs, I_imp_ind, A_imp_ind):
    img = np.ascontiguousarray(image_outputs, dtype=np.float32)
    aud = np.ascontiguousarray(audio_outputs, dtype=np.float32)
    I_imp = np.asarray(I_imp_ind).astype(np.int64)
    A_imp = np.asarray(A_imp_ind).astype(np.int64)
    in_maps = []
    for c in range(NCORES):
        base = c * SHARD
        in_maps.append({
            "img_full": img,
            "aud_full": aud,
            "img_loc": np.ascontiguousarray(img[base:base + SHARD]),
            "aud_loc": np.ascontiguousarray(aud[base:base + SHARD]),
            "iidx": _prep_idx(I_imp[base:base + SHARD]),
            "aidx": _prep_idx(A_imp[base:base + SHARD]),
        })
    return in_maps


def kernel(image_outputs, audio_outputs, I_imp_ind, A_imp_ind):
    from concourse import bass_utils

    nc = _get_nc()
    in_maps = make_in_maps(image_outputs, audio_outputs, I_imp_ind, A_imp_ind)
    res = bass_utils.run_bass_kernel_spmd(nc, in_maps, list(range(NCORES))).results
    total = sum(float(r["partial"].sum(dtype=np.float64)) for r in res)
    return np.float32(total / N)


# revision 11
# speedup vs baseline: 1.0404x; 1.0404x over previous
"""DotLoss kernel for Trainium2, data-parallel over 8 NeuronCores.

loss = mean_i[ relu(1 + dot(img[I[i]], aud[i]) - dot(img[i], aud[i]))
             + relu(1 + dot(img[i], aud[A[i]]) - dot(img[i], aud[i])) ]

Each core handles N/8 = 4096 rows: local rows stream in via contiguous
HWDGE DMAs, impostor rows via SWDGE dma_gather from the full (replicated)
embedding tables in device DRAM. Row dots are fused multiply+reduce ops on
the vector engine. Each core emits a [128,1] partial hinge-sum; the host
sums partials and divides by N.

Row->-(partition,slot) mapping: local chunk k row = k*CHUNK + p*SLOTS + c
lands at partition p, slot c (keeps local DMA fully contiguous per
partition). dma_gather position i lands at partition i%128, slot i//128, so
the host permutes each chunk's impostor indices with i = c*128 + p. The
summed loss is permutation-invariant, so only the triple alignment matters.
"""

import numpy as np

N, D = 32768, 512
NCORES = 8
SHARD = N // NCORES          # 4096 rows per core
P = 128
CHUNK = 512                  # rows per pipelined chunk
NCHUNK = SHARD // CHUNK
SLOTS = CHUNK // P           # [128, SLOTS, D] tiles per chunk
TSLOTS = SHARD // P          # accumulator columns per core
IC = CHUNK // 16             # idx columns per chunk in the wrapped layout

_CACHE = {}


def _build_nc():
    import concourse.bacc as bacc
    import concourse.mybir as mybir
    import concourse.tile as tile
    from contextlib import ExitStack

    fp32 = mybir.dt.float32
    i16 = mybir.dt.int16

    nc = bacc.Bacc("TRN2")
    img_full = nc.dram_tensor("img_full", [N, D], fp32, kind="ExternalInput")
    aud_full = nc.dram_tensor("aud_full", [N, D], fp32, kind="ExternalInput")
    img_loc = nc.dram_tensor("img_loc", [SHARD, D], fp32, kind="ExternalInput")
    aud_loc = nc.dram_tensor("aud_loc", [SHARD, D], fp32, kind="ExternalInput")
    iidx = nc.dram_tensor("iidx", [P, SHARD // 16], i16, kind="ExternalInput")
    aidx = nc.dram_tensor("aidx", [P, SHARD // 16], i16, kind="ExternalInput")
    partial = nc.dram_tensor("partial", [P, 1], fp32, kind="ExternalOutput")

    # [NCHUNK, P, SLOTS*D]: per partition one contiguous 8KB run per chunk,
    # so the HWDGE emits large descriptors instead of per-row 2KB ones.
    img_loc_r = img_loc.rearrange("(k p c) d -> k p (c d)", p=P, c=SLOTS)
    aud_loc_r = aud_loc.rearrange("(k p c) d -> k p (c d)", p=P, c=SLOTS)

    mult = mybir.AluOpType.mult
    add = mybir.AluOpType.add
    amax = mybir.AluOpType.max

    with ExitStack() as ctx:
        tc = ctx.enter_context(tile.TileContext(nc))
        io = ctx.enter_context(tc.tile_pool(name="io", bufs=4))
        idxp = ctx.enter_context(tc.tile_pool(name="idxp", bufs=1))
        acc = ctx.enter_context(tc.tile_pool(name="acc", bufs=1))
        scr = ctx.enter_context(tc.tile_pool(name="scr", bufs=6))

        iidx_sb = idxp.tile([P, SHARD // 16], i16, tag="iidx")
        nc.sync.dma_start(out=iidx_sb[:], in_=iidx[:])
        aidx_sb = idxp.tile([P, SHARD // 16], i16, tag="aidx")
        nc.sync.dma_start(out=aidx_sb[:], in_=aidx[:])

        anchor = acc.tile([P, TSLOTS], fp32, tag="anchor")
        iimp = acc.tile([P, TSLOTS], fp32, tag="iimp")
        aimp = acc.tile([P, TSLOTS], fp32, tag="aimp")

        for k in range(NCHUNK):
            gi = io.tile([P, SLOTS, D], fp32, tag="gi")
            nc.gpsimd.dma_gather(
                out_ap=gi[:],
                in_ap=img_full[:],
                idxs_ap=iidx_sb[:, k * IC:(k + 1) * IC],
                num_idxs=CHUNK,
                num_idxs_reg=CHUNK,
                elem_size=D,
            )
            ga = io.tile([P, SLOTS, D], fp32, tag="ga")
            nc.gpsimd.dma_gather(
                out_ap=ga[:],
                in_ap=aud_full[:],
                idxs_ap=aidx_sb[:, k * IC:(k + 1) * IC],
                num_idxs=CHUNK,
                num_idxs_reg=CHUNK,
                elem_size=D,
            )
            li = io.tile([P, SLOTS, D], fp32, tag="li")
            nc.sync.dma_start(out=li[:].rearrange("p c d -> p (c d)"),
                              in_=img_loc_r[k])
            la = io.tile([P, SLOTS, D], fp32, tag="la")
            nc.sync.dma_start(out=la[:].rearrange("p c d -> p (c d)"),
                              in_=aud_loc_r[k])
            for c in range(SLOTS):
                g = k * SLOTS + c
                for dst, a, b in ((anchor, li, la), (iimp, gi, la), (aimp, li, ga)):
                    pr = scr.tile([P, D], fp32, tag="pr")
                    nc.vector.scalar_tensor_tensor(
                        out=pr[:], in0=a[:, c], scalar=1.0, in1=b[:, c],
                        op0=mult, op1=mult,
                        accum_out=dst[:, g:g + 1],
                    )

        diff = acc.tile([P, 2 * TSLOTS], fp32, tag="diff")
        nc.vector.tensor_sub(diff[:, 0:TSLOTS], iimp[:], anchor[:])
        nc.vector.tensor_sub(diff[:, TSLOTS:], aimp[:], anchor[:])
        hout = acc.tile([P, 2 * TSLOTS], fp32, tag="hout")
        nc.vector.tensor_scalar(
            out=hout[:], in0=diff[:], scalar1=1.0, scalar2=0.0,
            op0=add, op1=amax,
        )
        psum_t = acc.tile([P, 1], fp32, tag="psum")
        nc.vector.tensor_reduce(
            out=psum_t[:], in_=hout[:], axis=mybir.AxisListType.X, op=add,
        )
        nc.sync.dma_start(out=partial[:], in_=psum_t[:])

    nc.compile()
    return nc


def _get_nc():
    if "nc" not in _CACHE:
        _CACHE["nc"] = _build_nc()
    return _CACHE["nc"]


def _prep_idx(imp_core):
    """Wrap one core's impostor indices into the dma_gather SBUF layout.

    Per chunk, gather position i = c*128 + p must fetch the impostor of
    local row p*SLOTS + c. The wrapped tile stores gather position j at
    [j % 16, j // 16], replicated across the 8 GPSIMD partition groups.
    """
    g = imp_core.reshape(NCHUNK, P, SLOTS)
    gi = np.transpose(g, (0, 2, 1)).reshape(NCHUNK, CHUNK)   # [k, c*P + p]
    w = gi.reshape(NCHUNK, IC, 16)
    w = np.transpose(w, (2, 0, 1)).reshape(16, SHARD // 16)  # [q, (k s)]
    return np.ascontiguousarray(np.tile(w, (8, 1)).astype(np.int16))


def make_in_maps(image_outputs, audio_outputs, I_imp_ind, A_imp_ind):
    img = np.ascontiguousarray(image_outputs, dtype=np.float32)
    aud = np.ascontiguousarray(audio_outputs, dtype=np.float32)
    I_imp = np.asarray(I_imp_ind).astype(np.int64)
    A_imp = np.asarray(A_imp_ind).astype(np.int64)
    in_maps = []
    for c in range(NCORES):
        base = c * SHARD
        in_maps.append({
            "img_full": img,
            "aud_full": aud,
            "img_loc": np.ascontiguousarray(img[base:base + SHARD]),
            "aud_loc": np.ascontiguousarray(aud[base:base + SHARD]),
            "iidx": _prep_idx(I_imp[base:base + SHARD]),
            "aidx": _prep_idx(A_imp[base:base + SHARD]),
        })
    return in_maps


def kernel(image_outputs, audio_outputs, I_imp_ind, A_imp_ind):
    from concourse import bass_utils

    nc = _get_nc()
    in_maps = make_in_maps(image_outputs, audio_outputs, I_imp_ind, A_imp_ind)
    res = bass_utils.run_bass_kernel_spmd(nc, in_maps, list(range(NCORES))).results
    total = sum(float(r["partial"].sum(dtype=np.float64)) for r in res)
    return np.float32(total / N)
